# revision 1
# baseline (speedup 1.0000x reference)
"""Trainium2 Bass kernel for nn_CombinedGraphLoss (graph-loss over 8192x8192 adj).

loss = sum((A - decay)^2) + 0.1*sum|A - mean4(A)| + 0.001*sum(A^2)
with A = D^-1/2 relu(adj) D^-1/2, decay = exp(-0.1|i-j|).

Strategy (8 cores, row-sharded, full inputs per core):
  - each core gets relu(its 1024-row shard + 1 halo row each side) pre-converted
    to bf16 on the host (halves HBM traffic; relu exact on host; error budget
    validated against a float64/numpy emulation at ~1e-6 relative)
  - both passes use the same 9 overlapping 128-row tiles (stride 126); tiles
    0/1 stay cached in SBUF between passes
  - pass1: row sums d, split ACT Copy(accum_out) / DVE reduce halves
  - AllGather d -> dinv = exp(-0.5*ln(d+eps)) (avoids the inaccurate Rsqrt)
  - pass2: A1 = adj*dinv_i on ACT (bf16), A = A1*colfac on DVE into a
    zero-padded tile; stencil t = A - 0.25*(up+down+left+right) built on PE
    via 3 matmuls per 512-col chunk (tridiagonal lhsT built via iota);
    |t| row-sums via ACT Abs(accum_out) from PSUM; sum A^2 and the decay band
    sum A*decay via DVE scalar_tensor_tensor(accum_out) (band slice is a
    dynamic AP at pid*1024+r0-relative offset into the padded A tile).
  - decay terms decomposed: sum(A-decay)^2 = sumA^2 - 2*sum(A*decay) + sum(decay^2);
    sum decay^2 is analytic on host; A*decay only inside |i-j|<=1088 band
    (decay==0 exactly in fp32 outside |i-j|>1039).
  - host applies row-ownership masks (overlap tiles) and reduces in float64.

The wait-legalization passes below work around this toolchain's walrus, which
rejects instructions carrying more than one semaphore wait and miscompiles
EVENT_SEMAPHORE_RANGE_CLEAR.
"""

import numpy as np

import concourse.bass as bass
import concourse.mybir as mybir
from concourse import tile
from concourse.bass_utils import run_bass_kernel_spmd

from collections import defaultdict
def _facts_union(a, b):
    # facts: dict sem_id -> max value known reached
    for s, v in b.items():
        if a.get(s, -1) < v:
            a[s] = v
    return a


def strip_redundant_waits(nc, verbose=False):
    insts = []
    for bb in nc.m.functions[0].blocks:
        insts.extend(bb.instructions)

    # classify sems: updated by exactly one engine-proc (in-order) or not
    sem_updaters = defaultdict(set)
    for ins in insts:
        si = ins.sync_info
        if si is None:
            continue
        eng = getattr(ins, "engine", None)
        is_dma = type(ins).__name__ == "InstDMACopy"
        proc = ("dma", getattr(ins, "queue", "")) if is_dma else ("eng", str(eng))
        for u in si.on_update:
            sem_updaters[u.id].add(proc)
    inorder_sem = {
        s: next(iter(p))
        for s, p in sem_updaters.items()
        if len(p) == 1 and next(iter(p))[0] == "eng"
    }

    # walk in emission order, tracking per-proc facts and per-sem crossing facts
    proc_facts = defaultdict(dict)          # proc -> facts
    sem_cum = defaultdict(int)              # sem -> cumulative value
    sem_cross = defaultdict(list)           # sem -> [(cum_after, facts)]
    n_stripped = 0
    max_left = 0

    for ins in insts:
        si = ins.sync_info
        if si is None:
            continue
        eng = getattr(ins, "engine", None)
        is_dma = type(ins).__name__ == "InstDMACopy"
        proc = ("dma", getattr(ins, "queue", "")) if is_dma else ("eng", str(eng))
        in_order = not is_dma

        def wait_facts(w):
            # facts implied by "sem w.id >= w.value" holding
            f = {w.id: w.wait_value}
            if w.id in inorder_sem:
                for cum, facts in sem_cross[w.id]:
                    if cum >= w.wait_value:
                        _facts_union(f, facts)
                        break
            return f

        waits = list(si.on_wait)
        if len(waits) > 1:
            base = dict(proc_facts[proc]) if in_order else {}
            # engine-sem waits are always kept; other waits are dropped when
            # implied by program order + the kept engine-sem waits
            for w in waits:
                if w.id in inorder_sem:
                    _facts_union(base, wait_facts(w))
            keep = []
            drop = []
            for w in waits:
                if w.id not in inorder_sem and base.get(w.id, -1) >= w.wait_value:
                    drop.append(w)
                else:
                    keep.append(w)
            if drop:
                n_stripped += len(drop)
                from concourse import mybir

                ins.sync_info = mybir.SyncInfo(
                    on_wait=keep, on_update=list(si.on_update)
                )
                si = ins.sync_info
            waits = keep
        max_left = max(max_left, len(waits))

        # facts after this instruction completes
        myf = dict(proc_facts[proc]) if in_order else {}
        for w in waits:
            _facts_union(myf, wait_facts(w))
        for u in si.on_update:
            sem_cum[u.id] += u.update_value
            f = dict(myf)
            f[u.id] = sem_cum[u.id]
            sem_cross[u.id].append((sem_cum[u.id], f))
            if in_order:
                # own-sem value is part of this proc's program-order knowledge
                myf[u.id] = sem_cum[u.id]
        if in_order:
            proc_facts[proc] = myf

    if verbose:
        print(f"waitstrip: removed {n_stripped} waits, max remaining {max_left}")
    return n_stripped, max_left


def split_multi_waits(nc, verbose=False):
    """Rewrite instructions carrying >1 sync wait into a chain of same-engine
    NOPs each carrying one wait (in-order engine queues make this equivalent).
    Must run after strip_redundant_waits. DMACopy must already be single-wait.
    """
    from concourse import mybir

    n_split = 0
    for bb_w in nc.m.functions[0].blocks:
        il = bb_w.instructions
        i = 0
        while i < len(il):
            ins = il[i]
            si = ins.sync_info
            if si is not None and len(si.on_wait) > 1:
                # DMACopy here is SWDGE (engine=Pool): descriptor generation
                # runs in the Pool instruction stream, so a preceding Pool nop
                # legally gates it just like any compute instruction.
                waits = list(si.on_wait)
                extra, keep = waits[:-1], waits[-1:]
                for w in extra:
                    r = nc.engines[ins.engine].nop()
                    # pull the freshly appended nop out of whichever bb got it
                    nop_ins = r.ins
                    removed = False
                    for bb2 in nc.m.functions[0].blocks:
                        il2 = bb2.instructions
                        if il2 and il2[-1] is nop_ins:
                            il2.pop()
                            removed = True
                            break
                    assert removed, "could not locate appended nop"
                    nop_ins.sync_info = mybir.SyncInfo(on_wait=[w], on_update=[])
                    il.insert(i, nop_ins)
                    i += 1
                    n_split += 1
                ins.sync_info = mybir.SyncInfo(
                    on_wait=keep, on_update=list(si.on_update)
                )
            i += 1
    if verbose:
        print(f"waitstrip: split {n_split} waits onto nops")
    return n_split


def drop_broken_range_clear(nc, verbose=False):
    """This walrus snapshot miscompiles EVENT_SEMAPHORE_RANGE_CLEAR ("ISA
    wrong length"). It only matters for re-executing an already-loaded NEFF
    with dirty semaphores; drop it (verified empirically with back-to-back
    executions)."""
    n = 0
    for bb_w in nc.m.functions[0].blocks:
        il = bb_w.instructions
        for i in range(len(il) - 1, -1, -1):
            ins = il[i]
            if type(ins).__name__ == "InstISA" and getattr(ins, "isa_opcode", 0) == 176:
                del il[i]
                n += 1
    if verbose:
        print(f"waitstrip: dropped {n} EVENT_SEMAPHORE_RANGE_CLEAR")


def legalize_waits(nc, verbose=False):
    drop_broken_range_clear(nc, verbose=verbose)
    strip_redundant_waits(nc, verbose=verbose)
    split_multi_waits(nc, verbose=verbose)
    bad = []
    for bb_w in nc.m.functions[0].blocks:
        for ins in bb_w.instructions:
            si = ins.sync_info
            if si is not None and len(si.on_wait) > 1:
                bad.append(ins.name)
    assert not bad, f"instructions still multi-wait: {bad}"


N = 8192
NC = 8
SH = N // NC          # 1024 rows per core
LR = SH + 2           # local rows incl halos = 1026
ALPHA = 0.1
LAM = 0.1
GAMMA = 0.001

BW = 2304             # band width (covers |i-j| <= 1088 for every tile row)
PAD = 1152            # zero padding on each side of A_pad
APW = N + 2 * PAD     # 10496
CB = PAD              # first real column inside A_pad
R0S = [126 * k for k in range(8)] + [LR - 128]   # pass2 tile starts (local rows)
NT2 = len(R0S)

f32 = mybir.dt.float32
bf16 = mybir.dt.bfloat16
i32 = mybir.dt.int32
Alu = mybir.AluOpType
Act = mybir.ActivationFunctionType
X = mybir.AxisListType.X

# accumulator column layout in the [128, 64] f32 output
SM_COL = 0     # 36 cols: tile k quarter q -> 4k+q, rows 0..125
A2_COL = 36    # 9 cols: tile k, rows 0..127
BD_COL = 45    # 9 cols: tile k, rows 0..127
A2B_COL = 54   # 9 cols: ACT-half of sum A^2


def _build_nc():
    nc = bass.Bass(num_devices=NC)
    adj_in = nc.dram_tensor("adj_sh", [LR, N], bf16, kind="ExternalInput")
    res_out = nc.dram_tensor("res", [128, 64], f32, kind="ExternalOutput")

    with tile.TileContext(nc) as tc:
        with (
            tc.tile_pool(name="const", bufs=1) as cp,
            tc.tile_pool(name="dram", bufs=1, space="DRAM") as dram,
            tc.tile_pool(name="io", bufs=2) as iop,
            tc.tile_pool(name="a1p", bufs=2) as a1p,
            tc.tile_pool(name="apad", bufs=1) as apadp,
            tc.tile_pool(name="scr", bufs=1) as scrp,
            tc.tile_pool(name="ps", bufs=1, space="PSUM") as psp,
        ):
            acc = cp.tile([128, 64], f32)
            nc.vector.memset(acc[:], 0.0)
            epsb = cp.tile([128, 1], f32)
            nc.vector.memset(epsb[:], 1e-10)

            apads = [apadp.tile([128, APW], bf16, tag=f"apad{i}", name=f"apad{i}") for i in range(2)]
            for a_t in apads:
                nc.gpsimd.memset(a_t[:, 0:PAD], 0.0)
                nc.gpsimd.memset(a_t[:, PAD + N : APW], 0.0)
            psums = [psp.tile([128, 2048], f32, tag=f"ps{i}", name=f"ps{i}") for i in range(2)]

            # ---- stencil lhsT matrices: Mv[p,l] = d(p,l+1) -0.25 d(p,l) -0.25 d(p,l+2)
            Mv = cp.tile([128, 126], bf16)
            NI = cp.tile([128, 126], bf16)
            idx = cp.tile([128, 126], i32)
            nc.gpsimd.iota(idx[:], pattern=[[-1, 126]], base=0, channel_multiplier=1)
            idxf = cp.tile([128, 126], f32)
            nc.gpsimd.tensor_copy(idxf[:], idx[:])
            vm1 = cp.tile([128, 126], f32)
            nc.vector.tensor_scalar(vm1[:], idxf[:], 1.0, None, Alu.subtract)  # p-l-1
            vab = cp.tile([128, 126], f32)
            vneg = cp.tile([128, 126], f32)
            nc.vector.tensor_scalar(vneg[:], vm1[:], -1.0, None, Alu.mult)
            nc.vector.tensor_max(vab[:], vm1[:], vneg[:])                      # |p-l-1|
            near = cp.tile([128, 126], f32)
            nc.vector.tensor_scalar(near[:], vab[:], 1.0, None, Alu.is_le)     # |.|<=1
            ctr = cp.tile([128, 126], f32)
            nc.vector.tensor_scalar(ctr[:], vab[:], 0.0, None, Alu.is_equal)   # ==0
            near4 = cp.tile([128, 126], f32)
            nc.vector.tensor_scalar(near4[:], near[:], 0.25, None, Alu.mult)
            ctr125 = cp.tile([128, 126], f32)
            nc.vector.tensor_scalar(ctr125[:], ctr[:], 1.25, None, Alu.mult)
            nc.vector.tensor_sub(Mv[:], ctr125[:], near4[:])
            nc.vector.tensor_scalar(NI[:], ctr[:], -0.25, None, Alu.mult)

            # ---- decay band constant: D[p,u] = exp(-0.1*|1088 + p - u|)
            decayb = cp.tile([128, BW], bf16)
            bidx = scrp.tile([128, BW], i32, tag="junk", name="bidx")
            nc.gpsimd.iota(bidx[:], pattern=[[-1, BW]], base=1088, channel_multiplier=1)
            bidf = scrp.tile([128, BW], f32, tag="sabs", name="bidf")
            nc.gpsimd.tensor_copy(bidf[:], bidx[:])
            babs = scrp.tile([128, BW], f32, tag="junk", name="babs")
            bneg = a1p.tile([128, BW], f32, tag="A1", name="bneg")
            nc.vector.tensor_scalar(bneg[:], bidf[:], -1.0, None, Alu.mult)
            nc.vector.tensor_max(babs[:], bidf[:], bneg[:])
            nc.scalar.activation(decayb[:], babs[:], Act.Exp, scale=-ALPHA)

            # ---- pass 1: d = row sums of relu(adj) over all 1026 local rows
            # relu'd bf16 rows staged in the apad center (reused later by pass 2)
            # adj is relu'd host-side; pass1 is pure row sums, split ACT/DVE.
            # Tiles use the same stride-126 starts as pass2 (overlap rows get
            # identical full-row sums, so overlapping writes are benign) and
            # tiles 0/1 stay cached in SBUF for pass2.
            d_sb = cp.tile([128, 16], f32)
            nc.vector.memset(d_sb[:], 0.0)
            d_sb2 = cp.tile([128, 16], f32)
            nc.vector.memset(d_sb2[:], 0.0)
            cach0 = cp.tile([128, N], bf16)
            cach1 = cp.tile([128, N], bf16)
            for k, r0 in enumerate(R0S):
                if k == 0:
                    t = cach0
                elif k == 1:
                    t = cach1
                else:
                    t = iop.tile([128, N], bf16, tag="adj", name=f"p1t{k}")
                nc.gpsimd.dma_start(t[:], adj_in[r0 : r0 + 128, :])
                rl = apads[k % 2][:, CB : CB + N // 2]
                nc.scalar.activation(
                    rl, t[:, 0 : N // 2], Act.Copy, accum_out=d_sb[:, k : k + 1]
                )
                nc.vector.tensor_reduce(
                    d_sb2[:, k : k + 1], t[:, N // 2 : N], axis=X, op=Alu.add
                )
            d_tot = cp.tile([128, 16], f32)
            nc.vector.tensor_add(d_tot[:], d_sb[:], d_sb2[:])

            # ---- dinv_local = exp(-0.5*ln(d + 1e-10)) ; store to DRAM flat [1152]
            lnd = cp.tile([128, 16], f32)
            nc.scalar.activation(lnd[:, 0:9], d_tot[:, 0:9], Act.Ln, bias=epsb[:])
            dinv_sb = cp.tile([128, 16], f32)
            nc.scalar.activation(dinv_sb[:, 0:9], lnd[:, 0:9], Act.Exp, scale=-0.5)
            # ---- AllGather of own d (local rows 1..1024 = global shard rows).
            # dcore[L-1] = d(local row L); tiles overlap with identical values.
            dcore = dram.tile([1, SH], f32)
            nc.sync.dma_start(dcore[0:1, 0:127], d_tot[1:128, 0:1])
            for k in range(1, 8):
                r0 = R0S[k]
                nc.sync.dma_start(
                    dcore[0:1, r0 - 1 : r0 + 127], d_tot[:, k : k + 1]
                )
            nc.sync.dma_start(dcore[0:1, 897:1024], d_tot[0:127, 8:9])
            dglob = dram.tile([NC, SH], f32)
            nc.gpsimd.collective_compute(
                "AllGather",
                Alu.bypass,
                replica_groups=[list(range(NC))],
                ins=[dcore.opt()],
                outs=[dglob.opt()],
            )

            # ---- global dinv -> padded bf16 DRAM vector + colfac tile
            dg = cp.tile([128, 64], f32)
            nc.sync.dma_start(
                dg[:], dglob[:].rearrange("a b -> (a b)").rearrange("(p t) -> p t", p=128)
            )
            lng = cp.tile([128, 64], f32)
            nc.scalar.activation(lng[:], dg[:], Act.Ln, bias=epsb[:])
            dgi = cp.tile([128, 64], f32)
            nc.scalar.activation(dgi[:], lng[:], Act.Exp, scale=-0.5)
            dgib = cp.tile([128, 64], bf16)
            nc.vector.tensor_copy(dgib[:], dgi[:])
            dinv3 = dram.tile([1, APW], bf16)
            zpad = cp.tile([1, PAD], bf16)
            nc.vector.memset(zpad[:], 0.0)
            nc.sync.dma_start(dinv3[0:1, 0:PAD], zpad[0:1, :])
            nc.sync.dma_start(dinv3[0:1, PAD + N : APW], zpad[0:1, :])
            nc.sync.dma_start(
                dinv3[0:1, PAD : PAD + N].rearrange("o (p t) -> (o p) t", p=128),
                dgib[:],
            )
            colfac = cp.tile([128, N], bf16)
            nc.sync.dma_start(
                colfac[:], dinv3[0:1, PAD : PAD + N].to_broadcast((128, N))
            )

            # ---- pass 2
            pid = nc.vector.partition_id()
            for k, r0 in enumerate(R0S):
                if k == 0:
                    adj_t = cach0
                elif k == 1:
                    adj_t = cach1
                else:
                    adj_t = iop.tile([128, N], bf16, tag="adj", name=f"adj{k}")
                    nc.gpsimd.dma_start(adj_t[:], adj_in[r0 : r0 + 128, :])
                A1 = a1p.tile([128, N], bf16, tag="A1", name=f"A1_{k}")
                nc.gpsimd.tensor_scalar(
                    A1[:], adj_t[:], dinv_sb[:, k : k + 1], None, Alu.mult
                )
                Apad = apads[k % 2]
                nc.vector.tensor_tensor(
                    Apad[:, CB : CB + N], A1[:], colfac[:], Alu.mult
                )

                # stencil: t = A -0.25*(up+down+left+right) built on PE
                for q in range(4):
                    ps = psums[q % 2]
                    for cc in range(4):
                        c = 4 * q + cc
                        col = CB + 512 * c
                        out_ap = ps[0:126, 512 * cc : 512 * cc + 512]
                        nc.tensor.matmul(
                            out_ap, Mv[:], Apad[:, col : col + 512],
                            start=True, stop=False,
                        )
                        nc.tensor.matmul(
                            out_ap, NI[:], Apad[:, col - 1 : col + 511],
                            start=False, stop=False,
                        )
                        nc.tensor.matmul(
                            out_ap, NI[:], Apad[:, col + 1 : col + 513],
                            start=False, stop=True,
                        )
                    if q == 0:
                        nc.vector.memset(ps[0:126, 0:1], 0.0)
                    if q == 3:
                        nc.vector.memset(ps[0:126, 2047:2048], 0.0)
                    sabs = scrp.tile([126, 2048], bf16, tag="sabs", name=f"sabs{k}_{q}")
                    nc.scalar.activation(
                        sabs[:], ps[0:126, :], Act.Abs,
                        accum_out=acc[0:126, 4 * k + q : 4 * k + q + 1],
                    )

                # sum A^2 (row partials)
                sq = scrp.tile([128, N // 2], bf16, tag="junk", name=f"sq{k}")
                nc.vector.scalar_tensor_tensor(
                    sq[:],
                    Apad[:, CB : CB + N // 2],
                    1.0,
                    Apad[:, CB : CB + N // 2],
                    Alu.bypass,
                    Alu.mult,
                    accum_out=acc[:, A2_COL + k : A2_COL + k + 1],
                )
                sqb = scrp.tile([128, N // 2], bf16, tag="junk", name=f"sqb{k}")
                nc.scalar.activation(
                    sqb[:],
                    Apad[:, CB + N // 2 : CB + N],
                    Act.Square,
                    accum_out=acc[:, A2B_COL + k : A2B_COL + k + 1],
                )

                # band sum A*decay (row partials); dynamic slice by core id
                bscr = scrp.tile([128, BW], bf16, tag="junk", name=f"bscr{k}")
                nc.vector.scalar_tensor_tensor(
                    bscr[:],
                    Apad[:, bass.ds(pid * SH + (r0 + 63), BW)],
                    1.0,
                    decayb[:],
                    Alu.bypass,
                    Alu.mult,
                    accum_out=acc[:, BD_COL + k : BD_COL + k + 1],
                )

            acc2 = cp.tile([128, 64], f32)
            nc.vector.tensor_copy(acc2[:], acc[:])
            nc.sync.dma_start(res_out[:], acc2[:])

    legalize_waits(nc)
    nc.finalize()
    drop_broken_range_clear(nc)
    return nc


def _masks():
    """Row-ownership masks resolving overlap-tile double counting (per core)."""
    sm = np.zeros((NC, 128, 36), np.float64)
    rows = np.zeros((NC, 128, 9), np.float64)
    for c in range(NC):
        claimed_r = set()
        claimed_s = set()
        for k, r0 in enumerate(R0S):
            for p in range(128):
                L = r0 + p
                if 1 <= L <= 1024 and L not in claimed_r:
                    claimed_r.add(L)
                    rows[c, p, k] = 1.0
            for p in range(126):
                L = r0 + 1 + p           # stencil out row (local)
                g = SH * c - 1 + L       # global row
                if 1 <= L <= 1024 and 1 <= g <= N - 2 and L not in claimed_s:
                    claimed_s.add(L)
                    sm[c, p, 4 * k : 4 * k + 4] = 1.0
    return sm, rows


_SM_MASK, _ROW_MASK = _masks()


def _analytic_decay_sq():
    k = np.arange(1, N, dtype=np.float64)
    return N + 2.0 * np.sum((N - k) * np.exp(-2.0 * ALPHA * k))


_NC_CACHE = None


def kernel(adj):
    global _NC_CACHE
    adj = np.ascontiguousarray(np.asarray(adj), dtype=np.float32)
    assert adj.shape == (N, N)

    if _NC_CACHE is None:
        _NC_CACHE = _build_nc()
    nc = _NC_CACHE

    import ml_dtypes

    in_maps = []
    for c in range(NC):
        sl = np.zeros((LR, N), ml_dtypes.bfloat16)
        lo = SH * c - 1
        src_lo = max(lo, 0)
        src_hi = min(lo + LR, N)
        sl[src_lo - lo : src_hi - lo, :] = np.maximum(adj[src_lo:src_hi], 0).astype(ml_dtypes.bfloat16)
        in_maps.append({"adj_sh": sl})

    res = run_bass_kernel_spmd(nc, in_maps, core_ids=list(range(NC)))
    s_sm = 0.0
    s_a2 = 0.0
    s_bd = 0.0
    for c in range(NC):
        o = res.results[c]["res"].astype(np.float64)
        s_sm += float((o[:, SM_COL : SM_COL + 36] * _SM_MASK[c]).sum())
        s_a2 += float((o[:, A2_COL : A2_COL + 9] * _ROW_MASK[c]).sum())
        s_a2 += float((o[:, A2B_COL : A2B_COL + 9] * _ROW_MASK[c]).sum())
        s_bd += float((o[:, BD_COL : BD_COL + 9] * _ROW_MASK[c]).sum())

    d2 = _analytic_decay_sq()
    loss = (s_a2 - 2.0 * s_bd + d2) + LAM * s_sm + GAMMA * s_a2
    return np.array(loss, dtype=np.float32)

